# revision 38
# baseline (speedup 1.0000x reference)
"""Channel-attention module (CAM) kernel for Trainium2.

Reference computation (per batch b):
    a    = x[b].reshape(HW, C)                      # [4096, 512]
    aTa  = a.T @ a                                  # [512, 512]
    attn = softmax(aTa, axis=-1)
    y    = a @ attn                                 # [4096, 512]
    out[b] = gamma * y + x[b]

Sharding: data-parallel over batch B=16 across 8 NeuronCores (2 batches
per core), gamma replicated.  No collectives needed.

PE-bound kernel (~112us of matmul at 2.4GHz vs ~90us of DMA); the
schedule exists to keep the PE warm (HAM clock gate at 8/8) and gapless:

  warmup   14 throwaway f32r matmuls gated only on a gpsimd memset start
           right after the engine preamble; they hold the PE busy
           through the HAM activity window so the clock flips to 2.4GHz
           at ~11us (a cold PE runs at 1.2GHz) and cover the DMA ramp.
  fused b0 each arriving 128-row chunk runs its 4 symmetric-aTa matmuls
           (bf16: blocks at/above the diagonal only, rhs free dim
           512/384/256/128) AND its 4 PE transposes immediately; the
           chunk work (~780ns) slightly exceeds the Sync DMA issue
           cadence (~650ns), so the load phase has zero PE idle.
           Transposes read the bf16 cast tiles and evacuate (ACT) into
           one [128, CB, HW] bf16 aT tile via a strided copy.
  bridges  2-4 throwaway matmuls at each phase boundary keep the PE (and
           the HAM monitor) ticking while DVE/ACT drain evac backlogs.
  mirrors  lower aTa blocks = PE transposes of the upper ones,
           interleaved 2-per-chunk between other matmul work so the PE
           queue never camps on a mirror whose msrc copy is queued.
  softmax  folds gamma into the row normalizer and adds I so pass 2
           directly yields gamma*y + a = a @ (g*attn + I); emitted
           interleaved into adjacent matmul loops (strict-FIFO DVE/ACT
           would otherwise head-block on the exp ping-pong).
  p1(b1)   runs between mirrors(b0) and p2(b0), covering softmax(b0);
           its first 12 casts are pre-issued so the DVE backlog cannot
           stall it.  b1 input DMAs are emitted strictly after b0's on
           the Sync queue (interleaving would halve b0 delivery).
  p2       32 chunks x 4 bf16 matmuls vs attn' (LDWEIGHTS hides under
           the 512-wide streams); PSUM evac on DVE; out-DMA issued from
           the ACT HWDGE queue so input loads never queue behind output.
  tp(b1)   braided 1 transpose-chunk : 1 p2(b0|b1) chunk with 6-chunk
           lookahead -- a pure-transpose block does not tick the HAM
           activity monitor and gets the PE re-throttled to 1.2GHz.

Precision: pass 1 is insensitive (the ~HW-sized aTa diagonal towers over
off-diagonal entries, so softmax saturates for this operator).  aT (pass
2 stationary operand) and attn' are bf16; y accumulates in fp32 PSUM.
Measured rel err 4.6e-3 vs the 2e-2 gate (the bf16 rounding of a is
scale-free relative error, so per-element error stays ~1e-2 bounded).

Measured on trn2 (8 cores, axon): 129.0us HW exec (baseline 142.2us),
rel err 4.6e-3; PE gaps 3.4us total, one HAM ramp, cool-clock verified
(bf16 N=512 matmul spacing 216ns).
"""

import numpy as np

import concourse.bacc as bacc
import concourse.mybir as mybir
import concourse.tile as tile
from concourse.bass_utils import run_bass_kernel_spmd
from concourse.masks import make_identity

B, H, W, C = 16, 64, 64, 512
HW = H * W                      # 4096
NCORES = 8
BPC = B // NCORES               # batches per core
NT = HW // 128                  # 32 row-chunks of a
CB = C // 128                   # 4 column-blocks of C
F32 = mybir.dt.float32
F32R = mybir.dt.float32r
BF16 = mybir.dt.bfloat16


def build_bass():
    nc = bacc.Bacc("TRN2", target_bir_lowering=False, debug=False)
    x = nc.dram_tensor("x", [BPC, HW, C], F32, kind="ExternalInput").ap()
    gamma = nc.dram_tensor("gamma", [1], F32, kind="ExternalInput").ap()
    out = nc.dram_tensor("out", [BPC, HW, C], F32, kind="ExternalOutput").ap()

    with tile.TileContext(nc) as tc:
        with (
            tc.tile_pool(name="singles", bufs=1) as singles,
            tc.tile_pool(name="a", bufs=26) as a_pool,
            tc.tile_pool(name="at", bufs=1) as at_pool,
            tc.tile_pool(name="atasb", bufs=6) as atasb_pool,
            tc.tile_pool(name="attn", bufs=8) as attn_pool,
            tc.tile_pool(name="stats", bufs=16) as stats_pool,
            tc.tile_pool(name="ostage", bufs=6) as out_pool,
            tc.tile_pool(name="abf", bufs=38) as bf_pool,
            tc.tile_pool(name="psum", bufs=8, space="PSUM") as psum_pool,
        ):
            # PE warmup: depends only on one gpsimd memset, so it starts
            # right after the engine preamble barrier and keeps the PE
            # busy through a full HAM activity window -> clock flips to
            # 8/8 before the real pass-1 work begins.
            warm_f = singles.tile([128, 512], F32)
            nc.gpsimd.memset(warm_f, 0.0)
            warm = warm_f.bitcast(F32R)
            wps = psum_pool.tile([128, C], F32, tag="ps")
            for _ in range(14):
                nc.tensor.matmul(
                    wps, warm[:, :128], warm, start=True, stop=True
                )

            ident = singles.tile([128, 128], F32)
            make_identity(nc, ident)
            ident_r = singles.tile([128, 128], F32R)
            # on ACT, not DVE: DVE is strict FIFO and this copy waits on
            # gpsimd's make_identity — it would stall the pass-1 casts
            nc.scalar.copy(ident_r, ident)
            ident_b = singles.tile([128, 128], BF16)
            nc.scalar.copy(ident_b, ident)
            gam = singles.tile([128, 1], F32)
            nc.gpsimd.dma_start(out=gam, in_=gamma.to_broadcast((128, 1)))

            st = [dict() for _ in range(BPC)]   # per-batch tile state

            def alloc_at(b):
                # single [128, CB, HW] f32r tile: one strided evac per
                # transpose bank; pool bufs=1 so b1 reuses b0's buffer
                # once p2(b0) has consumed it.
                st[b]["at"] = at_pool.tile(
                    [128, CB, HW], BF16, tag="at", name="at"
                )

            def load_chunk(b, k):
                """Issue the input DMA for chunk k of batch b (Sync)."""
                s = st[b]
                ak = a_pool.tile([128, C], F32R, tag="a", name="a")
                nc.sync.dma_start(
                    out=ak,
                    in_=x[b, k * 128:(k + 1) * 128, :].bitcast(F32R),
                )
                s.setdefault("a", {})[k] = ak

            def cast_chunk(b, k, on_act=False):
                s = st[b]
                ab = bf_pool.tile([128, C], BF16, tag="abf", name="abf")
                if on_act:
                    nc.scalar.copy(ab, s["a"][k].bitcast(F32))
                else:
                    nc.vector.tensor_copy(ab, s["a"][k].bitcast(F32))
                s.setdefault("abf", {})[k] = ab

            def p1_chunk(b, k, fuse_tp=False):
                """4 pass-1 matmuls (bf16) for chunk k; load/cast must
                already be emitted."""
                s = st[b]
                if k == 0:
                    s["ata"] = [
                        psum_pool.tile([128, C], F32, tag="ps", name="ata")
                        for _ in range(CB)
                    ]
                ab = s["abf"][k]
                for cb in range(CB):
                    nc.tensor.matmul(
                        s["ata"][cb][:, cb * 128:C],
                        ab[:, cb * 128:(cb + 1) * 128],
                        ab[:, cb * 128:C],
                        start=(k == 0),
                        stop=(k == NT - 1),
                    )
                if fuse_tp:
                    tp_chunk(b, k)

            def bridge(n):
                """Throwaway warm matmuls that keep the PE busy (and the
                HAM activity monitor ticking) across a phase transition
                while DVE/ACT drain evacuation queues."""
                bps = psum_pool.tile([128, C], F32, tag="ps", name="bridge")
                for _ in range(n):
                    nc.tensor.matmul(
                        bps, warm[:, :128], warm, start=True, stop=True
                    )

            def tp_chunk(b, k):
                """4 PE transposes of chunk k -> one PSUM bank -> one
                strided evac into the [128, CB, HW] aT tile.

                The evac always runs on ACT: in the fused b0 loop the DVE
                already carries the bf16 cast (476ns) and cast+evac would
                exceed the ~890ns chunk cadence and stall the PE."""
                s = st[b]
                tp = psum_pool.tile([128, C], BF16, tag="ps", name="tp")
                for cb in range(CB):
                    nc.tensor.transpose(
                        tp[:, cb * 128:(cb + 1) * 128],
                        s["abf"][k][:, cb * 128:(cb + 1) * 128],
                        ident_b,
                    )
                dst = s["at"][:, :, k * 128:(k + 1) * 128]
                src = tp.rearrange("p (c w) -> p c w", c=CB)
                nc.scalar.copy(dst, src)

            def evac_msrc(b):
                """Stage mirror sources + evacuate diag+upper aTa,
                row-ordered so each aTa PSUM bank frees as early as
                possible (the next batch's aTa accumulators reuse them)."""
                s = st[b]
                s["msrc"] = {}
                s["asb"] = [
                    atasb_pool.tile([128, C], F32, tag="atasb", name="asb")
                    for _ in range(CB)
                ]
                eng = 0
                for db in range(CB):
                    # all readers of aTa row db, back to back
                    for cb in range(db + 1, CB):
                        m = atasb_pool.tile(
                            [128, 128], F32R, tag="msrc", name="msrc", bufs=8
                        )
                        if eng % 2 == 0:
                            nc.vector.tensor_copy(
                                m, s["ata"][db][:, cb * 128:(cb + 1) * 128]
                            )
                        else:
                            nc.scalar.copy(
                                m, s["ata"][db][:, cb * 128:(cb + 1) * 128]
                            )
                        eng += 1
                        s["msrc"][(cb, db)] = m
                    if db % 2 == 0:
                        nc.vector.tensor_copy(
                            s["asb"][db][:, db * 128:C],
                            s["ata"][db][:, db * 128:C],
                        )
                    else:
                        nc.scalar.copy(
                            s["asb"][db][:, db * 128:C],
                            s["ata"][db][:, db * 128:C],
                        )

            def mirrors(b, pairs):
                """Fill lower aTa blocks: (cb, db) = (db, cb)^T via PE."""
                s = st[b]
                for cb, db in pairs:
                    if True:
                        mir = psum_pool.tile(
                            [128, 128], F32R, tag="ps", name="mir"
                        )
                        nc.tensor.transpose(mir, s["msrc"][(cb, db)], ident_r)
                        if (cb + db) % 2 == 0:
                            nc.vector.tensor_copy(
                                s["asb"][cb][:, db * 128:(db + 1) * 128],
                                mir.bitcast(F32),
                            )
                        else:
                            nc.scalar.copy(
                                s["asb"][cb][:, db * 128:(db + 1) * 128],
                                mir.bitcast(F32),
                            )

            def softmax(b, cbs=range(CB)):
                s = st[b]
                s.setdefault("attn", [])
                for cb in cbs:
                    asb = s["asb"][cb]
                    negmax = stats_pool.tile([128, 1], F32, tag="st")
                    nc.vector.reduce_max(
                        negmax, asb, axis=mybir.AxisListType.X, negate=True
                    )
                    rowsum = stats_pool.tile([128, 1], F32, tag="st")
                    nc.scalar.activation(
                        asb,
                        asb,
                        mybir.ActivationFunctionType.Exp,
                        bias=negmax,
                        scale=1.0,
                        accum_out=rowsum,
                    )
                    grec = stats_pool.tile([128, 1], F32, tag="st")
                    nc.vector.reciprocal(grec, rowsum)
                    # fold gamma into the row normalizer: attn' = g/rowsum * E
                    nc.vector.tensor_scalar_mul(grec, grec, gam)
                    nc.vector.tensor_scalar_mul(asb, asb, grec)
                    # + I on the diagonal block so pass2 fuses the residual
                    nc.vector.tensor_add(
                        asb[:, cb * 128:(cb + 1) * 128],
                        asb[:, cb * 128:(cb + 1) * 128],
                        ident,
                    )
                    ar = attn_pool.tile([128, C], BF16, tag="attn")
                    if cb % 2 == 0:
                        nc.vector.tensor_copy(ar, asb)
                    else:
                        nc.scalar.copy(ar, asb)
                    s["attn"].append(ar)

            def p2_chunk(b, k):
                s = st[b]
                yp = psum_pool.tile([128, C], F32, tag="ps", name="yp")
                for cb in range(CB):
                    nc.tensor.matmul(
                        yp,
                        s["at"][:, cb, k * 128:(k + 1) * 128],
                        s["attn"][cb],
                        start=(cb == 0),
                        stop=(cb == CB - 1),
                    )
                o = out_pool.tile([128, C], F32, tag="o", name="o")
                nc.vector.tensor_copy(o, yp)
                # out-DMA on the ACT HWDGE queue: keeps the Sync queue
                # free for input loads (no FIFO cross-blocking).
                nc.scalar.dma_start(
                    out=out[b, k * 128:(k + 1) * 128, :], in_=o
                )

            # ---------------- braided two-batch schedule ----------------
            # PE order: fused p1+tp(b0) | bridge | mirrors(b0) | p1(b1)
            # (covers softmax(b0)) | p2(b0) start | mirrors(b1) | rest of
            # p2(b0) (covers softmax(b1), emitted interleaved to avoid
            # DVE FIFO head-blocking) | tp(b1) braided 1:4 with p2(b1)
            # (pure-transpose blocks don't tick the HAM activity monitor
            # and get the PE clock re-throttled).
            alloc_at(0)
            for k in range(NT):
                load_chunk(0, k)
                cast_chunk(0, k)
                p1_chunk(0, k, fuse_tp=True)
            # b1 loads strictly AFTER all b0 loads on the Sync queue: the
            # fused loop's chunk cadence is gated by Sync DMA issue
            # (~620ns per transfer) and interleaving b1 transfers would
            # halve b0 delivery and stall the PE.
            for j in range(24):
                load_chunk(1, j)
            # keep the PE busy while DVE/ACT drain the tp-evac tail
            bridge(2)
            # early b1 casts FIRST in the post-fused DVE/ACT queues so
            # p1(b1) can start while the aTa evac/mirror chain drains
            for j in range(6):
                cast_chunk(1, j, on_act=(j % 2 == 1))
            evac_msrc(0)
            mpairs = [(cb, db) for cb in range(CB) for db in range(cb)]
            for k in range(6):
                p1_chunk(1, k)
                # mirror transposes interleave between p1 chunks so the
                # PE queue never camps on a mirror whose msrc copy is
                # still behind the DVE/ACT backlog
                mirrors(0, mpairs[2 * k:2 * k + 2])
            for j in range(6, 12):
                cast_chunk(1, j, on_act=(j % 2 == 1))
            softmax(0)
            for k in range(6, NT):
                if k >= 24:
                    load_chunk(1, k)
                if k >= 12:
                    cast_chunk(1, k, on_act=(k % 2 == 1))
                p1_chunk(1, k)
            for k in range(3):
                p2_chunk(0, k)
            evac_msrc(1)
            alloc_at(1)
            for k in range(3, NT):
                p2_chunk(0, k)
                if k < 9:
                    mirrors(1, mpairs[2 * (k - 3):2 * (k - 3) + 2])
                # softmax(b1) interleaved: its DVE/ACT ops trickle in
                # between the p2 evacs instead of head-blocking them
                if k in (9, 12, 15, 18):
                    softmax(1, cbs=[(k - 9) // 3])
                # deep transpose lookahead across the b0->b1 handover:
                # the PE keeps transposing while the at-tile WAR and the
                # first aT evacuations drain
                if k >= 28:
                    tp_chunk(1, k - 28)
            for j in (4, 5):
                tp_chunk(1, j)
            for k in range(NT):
                if k + 6 < NT:
                    tp_chunk(1, k + 6)
                p2_chunk(1, k)

    nc.compile()
    return nc


_NC_CACHE = None


def _get_nc():
    global _NC_CACHE
    if _NC_CACHE is None:
        _NC_CACHE = build_bass()
    return _NC_CACHE


def make_in_maps(x: np.ndarray, gamma: np.ndarray):
    x = np.ascontiguousarray(np.asarray(x, dtype=np.float32)).reshape(B, HW, C)
    gamma = np.ascontiguousarray(np.asarray(gamma, dtype=np.float32)).reshape(1)
    return [
        {"x": x[i * BPC:(i + 1) * BPC], "gamma": gamma} for i in range(NCORES)
    ]


def kernel(x: np.ndarray, gamma: np.ndarray, _trace: bool = False, _tmpdir=None):
    nc = _get_nc()
    in_maps = make_in_maps(x, gamma)
    res = run_bass_kernel_spmd(
        nc, in_maps, list(range(NCORES)), trace=_trace, tmpdir=_tmpdir
    )
    outs = [np.asarray(res.results[i]["out"]) for i in range(NCORES)]
    full = np.concatenate(outs, axis=0).reshape(B, H, W, C)
    if _trace:
        return full, res
    return full
